# revision 4
# baseline (speedup 1.0000x reference)
"""ActionCoherenceLoss kernel for 8 Trainium2 NeuronCores.

reference:
    norm = ||x||_2 along D; h = x / max(norm, eps)
    diag_sim[b, l] = <h[b,l], h[b,l+1]>          (l = 0..L-2)
    out = 1 - mean(diag_sim)                      (f32 scalar)

Strategy:
  - Data-parallel over batch: core b handles x[b] ([L=4096, D=2048]).
  - Host: transpose to x^T [D, L], pad one zero row -> [D, L+1], cast bf16.
  - Device: for each 128-row block i, compute the near-diagonal Gram block
        G_i = X_blk^T @ X_blk'  in PSUM  ([128, 129], fp32 accum over 16
        feature chunks of 128).  diag(G_i)[p] = s_{128i+p} = ||x_l||^2,
        superdiag(G_i)[p] = c_{128i+p} = <x_l, x_{l+1}>.
    Extract both diagonals with a fused masked multiply+reduce on VectorE.
  - Host: combine s, c from all 8 cores in float64:
        diag_sim_l = c_l / (max(sqrt(s_l),eps) * max(sqrt(s_{l+1}),eps))
"""

import numpy as np
import ml_dtypes

B, L, D = 8, 4096, 2048
P = 128
NCHUNK = D // P                # 16 feature chunks
SLABS = 4                      # row-range pipeline granularity for input DMA
SLAB_ROWS = L // SLABS         # 1024
SLAB_COLS = SLAB_ROWS + 1      # 1025 (one row overlap / zero pad at the end)
NBLK = L // P                  # 32 Gram blocks per core
BLK_PER_SLAB = NBLK // SLABS   # 8
EPS = 1e-12

_cache = {}


def _build():
    import concourse.bass as bass
    import concourse.bacc as bacc
    import concourse.tile as tile
    from concourse import mybir

    nc = bacc.Bacc(
        "TRN2", target_bir_lowering=False, debug=False, num_devices=B
    )
    f32 = mybir.dt.float32
    bf16 = mybir.dt.bfloat16

    xt_d = nc.dram_tensor(
        "xt", [NCHUNK, SLABS, P, SLAB_COLS], bf16, kind="ExternalInput"
    ).ap()
    mk_d = nc.dram_tensor("mk", [P, 2 * (P + 1)], f32, kind="ExternalInput").ap()
    sc_d = nc.dram_tensor("sc", [P, 2 * NBLK], f32, kind="ExternalOutput").ap()

    with tile.TileContext(nc) as tc:
        with (
            tc.tile_pool(name="xin", bufs=1) as xin,
            tc.tile_pool(name="cst", bufs=1) as cst,
            tc.tile_pool(name="scr", bufs=4) as scr,
            tc.tile_pool(name="outp", bufs=1) as outp,
            tc.tile_pool(name="psum", bufs=8, space=bass.MemorySpace.PSUM) as psum,
        ):
            mk = cst.tile([P, 2 * (P + 1)], f32, name="mk_sb")
            nc.sync.dma_start(out=mk, in_=mk_d)
            sc = outp.tile([P, 2 * NBLK], f32, name="sc_sb")

            # Input tiles, DMA'd slab-major so early row blocks are ready
            # while later slabs stream in.
            xt = {}
            for j in range(SLABS):
                for k in range(NCHUNK):
                    t = xin.tile(
                        [P, SLAB_COLS], bf16, tag=f"xt_{k}_{j}", name=f"xt_{k}_{j}"
                    )
                    nc.sync.dma_start(out=t, in_=xt_d[k, j])
                    xt[(k, j)] = t

            for i in range(NBLK):
                j, m = divmod(i, BLK_PER_SLAB)
                m0 = m * P
                pb = psum.tile([P, P + 1], f32, tag="gram", name=f"gram_{i}")
                for k in range(NCHUNK):
                    t = xt[(k, j)]
                    nc.tensor.matmul(
                        pb,
                        t[:, m0 : m0 + P],          # lhsT: stationary
                        t[:, m0 : m0 + P + 1],      # rhs: moving
                        start=(k == 0),
                        stop=(k == NCHUNK - 1),
                    )
                for h in range(2):  # 0 -> diag (s), 1 -> superdiag (c)
                    tmp = scr.tile(
                        [P, P + 1], f32, tag="scr", name=f"scr_{i}_{h}"
                    )
                    col = h * NBLK + i
                    nc.vector.tensor_mul(
                        tmp, pb, mk[:, h * (P + 1) : (h + 1) * (P + 1)]
                    )
                    nc.vector.reduce_sum(
                        sc[:, col : col + 1], tmp, axis=mybir.AxisListType.X
                    )

            nc.sync.dma_start(out=sc_d, in_=sc)
    nc.compile()
    return nc


def _make_masks():
    mk = np.zeros((P, 2 * (P + 1)), np.float32)
    r = np.arange(P)
    mk[r, r] = 1.0                  # diag mask (cols 0..128)
    mk[r, (P + 1) + r + 1] = 1.0    # superdiag mask (cols 129..257)
    return mk


def _prep_inputs(x):
    """x: [B, L, D] float32 -> list of per-core input maps."""
    mk = _make_masks()
    in_maps = []
    for b in range(B):
        xt = np.zeros((D, L + 1), dtype=ml_dtypes.bfloat16)
        xt[:, :L] = np.ascontiguousarray(x[b].T).astype(ml_dtypes.bfloat16)
        slabs = np.empty((NCHUNK, SLABS, P, SLAB_COLS), dtype=ml_dtypes.bfloat16)
        for j in range(SLABS):
            sl = xt[:, SLAB_ROWS * j : SLAB_ROWS * j + SLAB_COLS]
            slabs[:, j] = sl.reshape(NCHUNK, P, SLAB_COLS)
        in_maps.append({"xt": slabs, "mk": mk})
    return in_maps


def _combine(results):
    total = 0.0
    for b in range(B):
        sc = np.asarray(results[b]["sc"], dtype=np.float64)  # [P, 2*NBLK]
        s = sc[:, :NBLK].T.reshape(-1)  # s_l at [l % P, l // P]
        c = sc[:, NBLK:].T.reshape(-1)
        n = np.maximum(np.sqrt(s), EPS)
        diag = c[: L - 1] / (n[: L - 1] * n[1:L])
        total += diag.sum()
    coherence = total / (B * (L - 1))
    return np.array(1.0 - coherence, dtype=np.float32)


def _run(x, trace=False):
    from concourse import bass_utils

    if "nc" not in _cache:
        _cache["nc"] = _build()
    nc = _cache["nc"]
    in_maps = _prep_inputs(np.asarray(x, dtype=np.float32))
    res = bass_utils.run_bass_kernel_spmd(
        nc, in_maps, core_ids=list(range(B)), trace=trace
    )
    return _combine(res.results), res


def kernel(hidden_states):
    out, _ = _run(hidden_states, trace=False)
    return out
